# revision 1
# baseline (speedup 1.0000x reference)
"""NetVLAD pooling kernel for Trainium2 (8 NeuronCores, data-parallel over B).

Math per token m (of B*T=256):  logits = r @ W.T + b ; a = softmax(logits, -1)
    v = a.T @ r - a.sum(0)[:, None] * centroids          (r: [N=2048, C=64], K=32)

Mapping (per core = 32 tokens):
  - GEMM1 (contract C, bias fused): lhsT = rT tiles [65, 128] where row 64 is
    constant 1.0; rhs = [W.T; b] [65, 32]. One matmul per 128-n chunk writes
    logits+b into one PSUM bank [128, 16, 32] per token.
  - softmax: one EXP (Scalar, PSUM f32 -> SBUF bf16), tensor_reduce over k
    (Vector), reciprocal, one bf16 multiply -> a [128, 16, 32] bf16.
  - GEMM2 (contract N, flipped): lhsT = rn chunk [128, 65] (col 64 = -1),
    rhs = a chunk [128, 32]. out = v.T [65, 32] per token, 16 tokens col-packed
    into one PSUM bank [65, 512]; row 64 = -sum_n(a).
  - No device epilogue: Scalar copies each vt bank to SBUF bf16, DMA to DRAM.
    Host applies v += (-asum) * centroids and transposes to [tok, K, C].
"""

import os
import sys

import numpy as np

sys.path.insert(0, "/opt/trn_rl_repo")

import ml_dtypes  # noqa: E402

import concourse.bass as bass  # noqa: E402
import concourse.tile as tile  # noqa: E402
from concourse import mybir  # noqa: E402
from concourse import dve_ops as _dvo  # noqa: E402
from concourse import dve_spec as _dsp  # noqa: E402
from concourse.bass_utils import run_bass_kernel_spmd  # noqa: E402
from concourse.dve_uop import DveOpSpec as _DveOpSpec  # noqa: E402


def _register_cumsum_mul_op():
    """Custom DVE op: out = cumsum(in0 * in1) over the free dim (fp32 scan).

    Fuses the beta-weighting multiply with the softmax-denominator reduce:
    per-chunk sums fall out as differences of the running sum at chunk
    boundaries. Registered into dve_ops.OPS at import so the per-NEFF ucode
    table generation picks it up."""
    name = "NETVLAD_CUMSUM_MUL_ANT"
    for o in _dvo.OPS:
        if o.name == name:
            return o
    spec = _dsp.Spec(
        body=_dsp.scan(_dsp.AluOp.ADD, _dsp.Src0 * _dsp.Src1),
        reference=lambda in0, in1, s0, s1, imm2: np.cumsum(
            (in0.astype(np.float32) * in1).reshape(in0.shape[0], -1), axis=1
        ).reshape(in0.shape),
    )
    row = _dvo._CUSTOM_DVE_ROW_BASE + len(_dvo.OPS)
    assert row < 0x20
    shas = {}
    for ver in ("v3", "v4"):
        uops = _dsp.lower(spec, ver=ver)
        s = _DveOpSpec(
            name=name, opcode=row, uops=uops, rd1_en=_dsp._has_src1(spec)
        )
        shas[ver] = s.sha(ver)
    op = _dvo.DveOp(name, spec, subdim=False, uops_sha=shas)
    _dvo.OPS.append(op)
    _dvo.CUSTOM_DVE_SPECS[name] = spec
    _dvo._SUB_OPCODE_FOR_NAME[name] = row
    return op


CUMSUM_MUL = _register_cumsum_mul_op()

B, T, N, C, K = 8, 32, 2048, 64, 32
NCORES = 8
TOK = (B * T) // NCORES  # 32 tokens per core
TPB = 4                  # tokens per DMA batch
NB = TOK // TPB          # 8 batches
NCH = N // 128           # 16 n-chunks per token
GRP = 16                 # tokens per vt PSUM bank
NG = TOK // GRP          # 2 groups
LAG = int(os.environ.get("NETVLAD_LAG", "6"))  # GEMM2 trails GEMM1 (hides softmax)

BF16 = mybir.dt.bfloat16
F32 = mybir.dt.float32

_CACHE = {}

_NO_SPLIT_TYPES = ("InstEventSemaphore",)


def _split_excess_waits(nc):
    """walrus' setupSyncWait refuses >1 sem wait on (at least) the TT-family
    structs. Hoist extra waits onto standalone InstEventSemaphore ops."""
    for f in nc.m.functions:
        for blk in f.blocks:
            out = []
            changed = False
            for inst in blk.instructions:
                si = getattr(inst, "sync_info", None)
                if (
                    si is not None
                    and si.on_wait
                    and len(si.on_wait) > 1
                    and type(inst).__name__ not in _NO_SPLIT_TYPES
                ):
                    for idx, w in enumerate(si.on_wait[:-1]):
                        out.append(
                            mybir.InstEventSemaphore(
                                name=f"{inst.name}_xw{idx}",
                                engine=inst.engine,
                                sync_info=mybir.SyncInfo(on_wait=[w], on_update=[]),
                            )
                        )
                    inst.sync_info = mybir.SyncInfo(
                        on_wait=[si.on_wait[-1]], on_update=si.on_update
                    )
                    changed = True
                out.append(inst)
            if changed:
                try:
                    blk.instructions[:] = out
                except TypeError:
                    blk.instructions = out


def _build_nc(split_waits=True):
    stage = int(os.environ.get("NETVLAD_STAGE", "3"))  # 1=G1+exp 2=+softmax 3=full
    amul_eng = os.environ.get("NETVLAD_AMUL", "vector")  # gpsimd|vector
    nc = bass.Bass()
    # rT on all 128 partitions (p = 64h + c, h = n-half) — DMAs covering 128
    # partitions give each queue 8 consecutive partitions spanning 2 SBUF
    # write-port groups (~25 B/ns); 65-partition transfers sit on one port
    # group and run at ~15 B/ns.
    RT = nc.declare_dram_parameter("RT", [NB, 128, TPB, N // 2], BF16, False)
    RN = nc.declare_dram_parameter("RN", [NB, 128, TPB, NCH, 65], BF16, False)
    WT2 = nc.declare_dram_parameter("WT2", [128, K], BF16, False)
    # bias b[k] tiled over the two half-bank regions; prefilled into PSUM so
    # GEMM1 accumulates logits+b directly (start=False) and softmax needs no
    # separate beta weighting
    B8 = nc.declare_dram_parameter("B8", [128, 2, NCH // 2, K], BF16, False)
    VO = nc.declare_dram_parameter("VO", [NG, 65, GRP * K], BF16, True)

    # group-ordered: all of bank A (rows 0:64) first, then bank B — EXP(pla)
    # can start once the first 8 matmuls retire
    g1_order = list(range(16))

    with tile.TileContext(nc) as tc:
        with (
            tc.tile_pool(name="singles", bufs=1) as singles,
            tc.tile_pool(name="rt", bufs=5) as rt_pool,
            tc.tile_pool(name="rn", bufs=5) as rn_pool,
            tc.tile_pool(name="e", bufs=3) as e_pool,
            tc.tile_pool(name="a", bufs=LAG + 3) as a_pool,
            tc.tile_pool(name="s", bufs=6) as s_pool,
            tc.tile_pool(name="o", bufs=2) as o_pool,
            tc.tile_pool(name="pl", bufs=3, space="PSUM") as pl_pool,
            tc.tile_pool(name="pv", bufs=2, space="PSUM") as pv_pool,
        ):
            wt2_sb = singles.tile([128, K], BF16)
            nc.sync.dma_start(out=wt2_sb[:], in_=WT2[:])
            b8_sb = singles.tile([128, 2, NCH // 2, K], BF16)
            nc.sync.dma_start(out=b8_sb[:], in_=B8[:])
            # dummy EXP with no DMA dependency: pulls the one-time
            # ACT_TABLE_LOAD (~1.3us) into the boot shadow instead of
            # delaying token 0's first real EXP
            warm = singles.tile([1, 2], F32, name="warm", tag="warm")
            nc.vector.memset(warm[:, 0:1], 0.0)
            nc.scalar.activation(
                warm[:, 1:2], warm[:, 0:1], mybir.ActivationFunctionType.Exp
            )

            rt_sb = [None] * NB
            rn_sb = [None] * NB
            pv = [None] * NG
            a_t = [None] * TOK
            e_t = [None] * (TOK // 2)
            pl_t = [None] * TOK

            def prefill(tok):
                # one 2-bank PSUM tile per token: group h's logits live in
                # quarter 2h (bank h) — the two PE row groups must not share
                # a bank. One Scalar copy (strided AP over both banks) writes
                # the bias b; G1 matmuls accumulate on top (start=False).
                # Issued PREF tokens ahead so the PE never waits on it.
                pl_t[tok] = pl_pool.tile(
                    [128, 4, NCH // 2, K], F32, name="pl_t", tag="pl_t"
                )
                nc.scalar.activation(
                    pl_t[tok][:, 0:4:2], b8_sb[:], mybir.ActivationFunctionType.Copy
                )

            def load_batch(bi, split=False):
                rt_sb[bi] = rt_pool.tile(
                    [128, TPB, N // 2], BF16, name="rt_t", tag="rt_t"
                )
                rn_sb[bi] = rn_pool.tile(
                    [128, TPB, NCH, 65], BF16, name="rn_t", tag="rn_t"
                )
                if split:
                    # per-token slices so the first G1 starts after ~0.26MB
                    # instead of the full 2.1MB batch; rt slices first (G1
                    # needs them), rn afterwards (G2 needs them much later)
                    for ti in range(TPB):
                        nc.sync.dma_start(
                            out=rt_sb[bi][:, ti], in_=RT[bi, :, ti]
                        )
                    for ti in range(TPB):
                        nc.sync.dma_start(
                            out=rn_sb[bi][:, ti], in_=RN[bi, :, ti]
                        )
                else:
                    nc.sync.dma_start(out=rt_sb[bi][:], in_=RT[bi])
                    nc.sync.dma_start(out=rn_sb[bi][:], in_=RN[bi])

            def gemm1_softmax(tok):
                bi, ti = tok // TPB, tok % TPB
                # two PSUM banks per token: one per PE row-group — same-bank
                # alternation across row groups is a fatal HW collision
                pl2 = pl_t[tok]
                for j in g1_order:
                    h, jj = j // 8, j % 8
                    nc.tensor.matmul(
                        pl2[:, 2 * h, jj, :],
                        rt_sb[bi][64 * h : 64 * h + 64, ti, 128 * jj : 128 * jj + 128],
                        wt2_sb[64 * h : 64 * h + 64, :],
                        start=False,
                        stop=True,
                        skip_group_check=True,
                        tile_position=(64 * h, 0),
                    )
                # token-pair softmax: EXP per token into half of a shared
                # pair tile; one reduce/recip/normalize per TWO tokens
                # (halves DVE op count and cross-engine semaphore hops)
                pair, sub = tok // 2, tok % 2
                if sub == 0:
                    e_t[pair] = e_pool.tile(
                        [128, 4, NCH // 2, K], BF16, name="e_t", tag="e_t"
                    )
                nc.scalar.activation(
                    e_t[pair][:, 2 * sub : 2 * sub + 2],
                    pl2[:, 0:4:2],
                    mybir.ActivationFunctionType.Exp,
                )
                if stage <= 1:
                    a_t[tok] = (e_t[pair], sub)
                    return
                if sub == 0:
                    return
                s = s_pool.tile([128, 4, NCH // 2], F32)
                nc.vector.tensor_reduce(
                    s[:], e_t[pair][:], axis=mybir.AxisListType.X,
                    op=mybir.AluOpType.add,
                )
                rs = s_pool.tile([128, 4, NCH // 2], F32)
                nc.vector.reciprocal(rs[:], s[:])
                a = a_pool.tile([128, 4, NCH // 2, K], BF16)
                nc.vector.tensor_mul(
                    a[:],
                    e_t[pair][:],
                    rs[:].unsqueeze(3).broadcast_to((128, 4, NCH // 2, K)),
                )
                a_t[2 * pair] = (a, 0)
                a_t[2 * pair + 1] = (a, 1)

            def gemm2(tok):
                bi, ti = tok // TPB, tok % TPB
                g, hi = tok // GRP, tok % GRP
                if stage < 3:
                    a_t[tok] = None
                    return
                if hi == 0:
                    pv[g] = pv_pool.tile([65, GRP, K], F32, name="pv_t", tag="pv_t")
                a_tile, sub = a_t[tok]
                for j in range(NCH):
                    nc.tensor.matmul(
                        pv[g][:, hi, :],
                        rn_sb[bi][:, ti, j, :],
                        a_tile[:, 2 * sub + j // 8, j % 8, :],
                        start=(j == 0),
                        stop=(j == NCH - 1),
                        skip_group_check=True,
                    )
                a_t[tok] = None
                if hi == GRP - 1:
                    vo = o_pool.tile([65, GRP * K], BF16, name="o_t", tag="o_t")
                    nc.scalar.activation(
                        vo[:], pv[g][:], mybir.ActivationFunctionType.Copy
                    )
                    nc.sync.dma_start(out=VO[g], in_=vo[:])

            PREF = 2
            for t in range(PREF):
                prefill(t)
            load_batch(0, split=True)
            load_batch(1)
            for tok in range(TOK + LAG):
                # G2 emitted before G1 each step (phase-shifts the PE stream
                # by one G1 block relative to the softmax producers)
                lag_tok = tok - LAG
                if lag_tok >= 0:
                    gemm2(lag_tok)
                if tok < TOK:
                    bi, ti = tok // TPB, tok % TPB
                    if ti == 0 and bi + 2 < NB:
                        load_batch(bi + 2)
                    if tok + PREF < TOK:
                        prefill(tok + PREF)
                    gemm1_softmax(tok)
    if split_waits:
        _split_excess_waits(nc)
    return nc


def _prep_core_inputs(r_core, WT2_h, B8_h):
    """r_core: [TOK, N, C] fp32 -> per-core input map."""
    bf = ml_dtypes.bfloat16
    # RT: [NB, 128, TPB, N//2]; partition p = 64h + c holds r[4b+t, 1024h+nn, c]
    r5 = r_core.reshape(NB, TPB, 2, N // 2, C)           # [b, t, h, nn, c]
    rt = np.ascontiguousarray(r5.transpose(0, 2, 4, 1, 3)).reshape(
        NB, 128, TPB, N // 2
    )
    # RN: [NB, 128, TPB, NCH, 65]; RN[b,p,t,j,:C] = r[4b+t, 128j+p, :], col 64 = -1
    r6 = r_core.reshape(NB, TPB, NCH, 128, C)            # [b, t, j, p, c]
    rn = np.empty((NB, 128, TPB, NCH, C + 1), dtype=np.float32)
    rn[..., :C] = r6.transpose(0, 3, 1, 2, 4)
    rn[..., C] = -1.0
    return {
        "RT": np.ascontiguousarray(rt.astype(bf)),
        "RN": np.ascontiguousarray(rn.astype(bf)),
        "WT2": WT2_h,
        "B8": B8_h,
    }


def kernel(R_seq, W, b, centroids):
    if "nc" not in _CACHE:
        _CACHE["nc"] = _build_nc()
    nc = _CACHE["nc"]

    bf = ml_dtypes.bfloat16
    WT = np.ascontiguousarray(W.astype(np.float32).T)            # [C, K]
    WT2_h = np.ascontiguousarray(np.concatenate([WT, WT], axis=0).astype(bf))
    B8_h = np.ascontiguousarray(
        np.broadcast_to(
            b.astype(np.float32)[None, None, None, :], (128, 2, NCH // 2, K)
        ).astype(bf)
    )

    r_all = np.asarray(R_seq, np.float32).reshape(NCORES, TOK, N, C)
    in_maps = [_prep_core_inputs(r_all[i], WT2_h, B8_h) for i in range(NCORES)]

    res = run_bass_kernel_spmd(
        nc,
        in_maps,
        list(range(NCORES)),
        trace=bool(int(os.environ.get("NETVLAD_TRACE", "0"))),
    )
    _CACHE["last_results"] = res

    cent = np.asarray(centroids, np.float32)             # [K, C]
    outs = []
    for i in range(NCORES):
        vo = np.asarray(res.results[i]["VO"], np.float32)
        vo = vo.reshape(NG, 65, GRP, K)
        vraw = vo[:, :C].transpose(0, 2, 3, 1).reshape(TOK, K, C)
        nasum = vo[:, C].reshape(TOK, K)  # = -sum_n a
        v = vraw + nasum[:, :, None] * cent[None]
        outs.append(v)
    out = np.stack(outs, axis=0).reshape(B, T, K, C).astype(np.float32)
    return out


if __name__ == "__main__":
    rng = np.random.default_rng(0)
    R = rng.normal(size=(B, T, N, C)).astype(np.float32)
    W_ = rng.normal(size=(K, C)).astype(np.float32) / 8.0
    b_ = (rng.normal(size=(K,)) * 0.01).astype(np.float32)
    cc = rng.normal(size=(K, C)).astype(np.float32)
    out = kernel(R, W_, b_, cc)
    print(out.shape, out.dtype)

